# revision 12
# baseline (speedup 1.0000x reference)
"""GroupQuantLinear int4 dequant + linear on 8 Trainium2 NeuronCores.

y = x @ W^T,  W = dequant(w_packed)*w_scale + w_bias  (group size 64)

Strategy (column-parallel): shard the 12288 output rows across 8 cores
(1536 each); x replicated. Per core:
  - contraction axis K=8192 split into 64 k-tiles of 128 partitions where
    partition p == group p and k-tile k == position k within each group.
    One extra k-tile holds the per-group sums of x matched against the
    bias rows, folding the bias term (sum_g bias[o,g]*xsum[t,g]) into the
    same PSUM accumulation.
  - int4 values are host-unpacked to uint8 (still 1B/elem in HBM); the
    dequant of each k-tile is ONE DVE multiply:
        wt[128 g, O] = nib_u8[128 g, O] * sT[128 g, O]   (-> bf16)
    with sT an honest fp32 tile (no broadcast needed: partition == group).
  - matmul in bf16 (fp32 PSUM accumulation), out [128 o, 512 t] per bank;
    12 o-tiles -> 2 passes of 6 PSUM banks.
"""
import os
import sys

for _p in ("/opt/trn_rl_repo",):
    if _p not in sys.path and os.path.isdir(_p):
        sys.path.insert(0, _p)

import numpy as np
import ml_dtypes

import concourse.bacc as bacc
import concourse.mybir as mybir
import concourse.tile as tile
from concourse import bass_utils

# ---- problem constants (hardcoded per contract) ----
B, S, IN_F, OUT_F = 4, 128, 8192, 12288
GS = 64                 # quant group size
NG = IN_F // GS         # 128 groups == partitions per k-tile
N_CORES = 8
O_CORE = OUT_F // N_CORES   # 1536
T = B * S                   # 512 tokens
NK = GS + 1                 # 64 nibble k-tiles + 1 bias k-tile
N_OPASS = 2                 # PSUM-capacity passes over output tiles


def host_prep_x(x):
    """x [B,S,I] fp32 -> xt [128, NK, T] bf16 (group-partition-major)."""
    x2 = x.reshape(T, NG, GS)
    xt = np.empty((NG, NK, T), dtype=np.float32)
    xt[:, 0] = x2.sum(axis=2, dtype=np.float64).T
    xt[:, 1:] = x2.transpose(1, 2, 0)
    return xt.astype(ml_dtypes.bfloat16)


def host_prep_w(w_packed, w_scale, w_bias):
    """-> per-core (wn [2,128,64,OH] u8, sT [128,Oc] f32, bT [128,Oc] bf16).

    Nibble unpack identical to the reference: group-position q = 16*blk+4*i+j
    comes from nibble i of packed word 4*blk+j. wn is pass-major and
    partition-major so weight DMAs read long contiguous per-partition lines.
    """
    p4 = w_packed.reshape(OUT_F, NG, 4, 4)
    nibs = np.stack([(p4 >> (4 * i)) & 0xF for i in range(4)], axis=-2)
    u = nibs.reshape(OUT_F, NG, GS).astype(np.uint8)        # [O, G, 64]
    OH = O_CORE // N_OPASS
    wns, sts, bts = [], [], []
    for c in range(N_CORES):
        sl = slice(c * O_CORE, (c + 1) * O_CORE)
        uc = u[sl].transpose(1, 2, 0)                        # [128, 64, Oc]
        wn = np.empty((N_OPASS, NG, GS, OH), dtype=np.uint8)
        for p in range(N_OPASS):
            wn[p] = uc[:, :, p * OH:(p + 1) * OH]
        wns.append(wn)
        sts.append(np.ascontiguousarray(w_scale[sl, :, 0].T))        # [128,Oc] f32
        bts.append(np.ascontiguousarray(w_bias[sl, :, 0].T).astype(ml_dtypes.bfloat16))
    return wns, sts, bts


def build():
    """Build the per-core bass program (identical on all cores)."""
    NOJ = O_CORE // 128
    OPP = NOJ // N_OPASS
    OH = OPP * 128

    # ramped DMA chunk sizes: small first chunks so the PE starts early
    XCH = [1, 2, 4, 6] + [8] * 6 + [4]    # x k-tile chunks (sum 65; xsum first)
    WCH = [2, 2, 4] + [8] * 7             # weight k-tile chunks per pass (sum 64)

    nc = bacc.Bacc("TRN2", target_bir_lowering=False)
    xt_d = nc.dram_tensor("xt", [NG, NK, T], mybir.dt.bfloat16, kind="ExternalInput")
    wn_d = nc.dram_tensor("wn", [N_OPASS, NG, GS, OH], mybir.dt.uint8,
                          kind="ExternalInput")
    st_d = nc.dram_tensor("st", [NG, O_CORE], mybir.dt.float32, kind="ExternalInput")
    bt_d = nc.dram_tensor("bt", [NG, O_CORE], mybir.dt.bfloat16, kind="ExternalInput")
    yt_d = nc.dram_tensor("yt", [O_CORE, T], mybir.dt.float32,
                          kind="ExternalOutput")

    with tile.TileContext(nc) as tc:
        with (
            tc.tile_pool(name="resident", bufs=1) as rpool,
            tc.tile_pool(name="nibs", bufs=4) as bpool,
            tc.tile_pool(name="wts", bufs=6) as wpool,
            tc.tile_pool(name="psum", bufs=8, space="PSUM") as ppool,
        ):
            # scale/bias on the scalar engine's queue; pass-0 scale half first
            st_s = rpool.tile([NG, O_CORE], mybir.dt.float32)
            for p in range(N_OPASS):
                nc.scalar.dma_start(st_s[:, p * OH:(p + 1) * OH],
                                    st_d[:, p * OH:(p + 1) * OH])
            bt_s = rpool.tile([NG, O_CORE], mybir.dt.bfloat16)
            nc.scalar.dma_start(bt_s[:], bt_d[:])
            # x on the gpsimd engine's queue, ramped chunks
            xt_s = rpool.tile([NG, NK, T], mybir.dt.bfloat16)
            k0 = 0
            for ch in XCH:
                nc.gpsimd.dma_start(xt_s[:, k0:k0 + ch, :], xt_d[:, k0:k0 + ch, :])
                k0 += ch

            for p in range(N_OPASS):
                oo = p * OH
                psums = [ppool.tile([128, T], mybir.dt.float32, tag="ps",
                                    name=f"ps_{p}_{j}")
                         for j in range(OPP)]
                # bias k-tile first: needs only xsum (xt idx 0) + bt
                for j in range(OPP):
                    nc.tensor.matmul(
                        psums[j][:],
                        bt_s[:, oo + j * 128: oo + (j + 1) * 128],
                        xt_s[:, 0, :],
                        start=True, stop=False)
                k0 = 0
                for ch in WCH:
                    # weights on the sync engine's queue, chunked
                    nt = bpool.tile([NG, ch, OH], mybir.dt.uint8, tag="nib",
                                    name=f"nib_{p}_{k0}")
                    nc.sync.dma_start(nt[:], wn_d[p, :, k0:k0 + ch, :])
                    for kk in range(ch):
                        k = k0 + kk
                        wt = wpool.tile([NG, OH], mybir.dt.bfloat16, tag="wt")
                        nc.vector.tensor_mul(wt[:], nt[:, kk, :],
                                             st_s[:, oo:oo + OH])
                        for j in range(OPP):
                            nc.tensor.matmul(
                                psums[j][:],
                                wt[:, j * 128:(j + 1) * 128],
                                xt_s[:, k + 1, :],
                                start=False, stop=(k == GS - 1))
                    k0 += ch
                for j in range(OPP):
                    ot = wpool.tile([128, T], mybir.dt.float32, tag="ot")
                    nc.vector.tensor_copy(ot[:], psums[j][:])
                    nc.scalar.dma_start(
                        yt_d[oo + j * 128: oo + (j + 1) * 128, :], ot[:])

    nc.compile()
    return nc


_NC_CACHE = None


def get_nc():
    global _NC_CACHE
    if _NC_CACHE is None:
        _NC_CACHE = build()
    return _NC_CACHE


def make_in_maps(x, w_packed, w_scale, w_bias):
    xt = host_prep_x(np.asarray(x, dtype=np.float32))
    wns, sts, bts = host_prep_w(np.asarray(w_packed), np.asarray(w_scale),
                                np.asarray(w_bias))
    return [{"xt": xt, "wn": wns[c], "st": sts[c], "bt": bts[c]}
            for c in range(N_CORES)]


def assemble_out(results):
    yt = np.concatenate([np.asarray(r["yt"]) for r in results], axis=0)
    return np.ascontiguousarray(yt.T).reshape(B, S, OUT_F).astype(np.float32)


def run(x, w_packed, w_scale, w_bias, trace=False, **kw):
    nc = get_nc()
    in_maps = make_in_maps(x, w_packed, w_scale, w_bias)
    res = bass_utils.run_bass_kernel_spmd(
        nc, in_maps, core_ids=list(range(N_CORES)), trace=trace, **kw)
    return assemble_out(res.results), res


def kernel(x, w_packed, w_scale, w_bias):
    out, _ = run(x, w_packed, w_scale, w_bias, trace=False)
    return out


# revision 14
# speedup vs baseline: 1.0315x; 1.0315x over previous
"""GroupQuantLinear int4 dequant + linear on 8 Trainium2 NeuronCores.

y = x @ W^T,  W = dequant(w_packed)*w_scale + w_bias  (group size 64)

Strategy (column-parallel): shard the 12288 output rows across 8 cores
(1536 each); x replicated. Per core:
  - contraction axis K=8192 split into 64 k-tiles of 128 partitions where
    partition p == group p and k-tile k == position k within each group.
    One extra k-tile holds the per-group sums of x matched against the
    bias rows, folding the bias term (sum_g bias[o,g]*xsum[t,g]) into the
    same PSUM accumulation.
  - int4 values are host-unpacked to uint8 (still 1B/elem in HBM); the
    dequant of each k-tile is ONE DVE multiply:
        wt[128 g, O] = nib_u8[128 g, O] * sT[128 g, O]   (-> bf16)
    with sT an honest fp32 tile (no broadcast needed: partition == group).
  - matmul in bf16 (fp32 PSUM accumulation), out [128 o, 512 t] per bank;
    12 o-tiles -> 2 passes of 6 PSUM banks.
"""
import os
import sys

for _p in ("/opt/trn_rl_repo",):
    if _p not in sys.path and os.path.isdir(_p):
        sys.path.insert(0, _p)

import numpy as np
import ml_dtypes

import concourse.bacc as bacc
import concourse.mybir as mybir
import concourse.tile as tile
from concourse import bass_utils

# ---- problem constants (hardcoded per contract) ----
B, S, IN_F, OUT_F = 4, 128, 8192, 12288
GS = 64                 # quant group size
NG = IN_F // GS         # 128 groups == partitions per k-tile
N_CORES = 8
O_CORE = OUT_F // N_CORES   # 1536
T = B * S                   # 512 tokens
NK = GS + 1                 # 64 nibble k-tiles + 1 bias k-tile
N_OPASS = 2                 # PSUM-capacity passes over output tiles


def host_prep_x(x):
    """x [B,S,I] fp32 -> xt [128, NK, T] bf16 (group-partition-major)."""
    x2 = x.reshape(T, NG, GS)
    xt = np.empty((NG, NK, T), dtype=np.float32)
    xt[:, 0] = x2.sum(axis=2, dtype=np.float64).T
    xt[:, 1:] = x2.transpose(1, 2, 0)
    return xt.astype(ml_dtypes.bfloat16)


def host_prep_w(w_packed, w_scale, w_bias):
    """-> per-core (wn [2,128,64,OH] u8, sT [128,Oc] f32, bT [128,Oc] bf16).

    Nibble unpack identical to the reference: group-position q = 16*blk+4*i+j
    comes from nibble i of packed word 4*blk+j. wn is pass-major and
    partition-major so weight DMAs read long contiguous per-partition lines.
    """
    p4 = w_packed.reshape(OUT_F, NG, 4, 4)
    nibs = np.stack([(p4 >> (4 * i)) & 0xF for i in range(4)], axis=-2)
    u = nibs.reshape(OUT_F, NG, GS).astype(np.uint8)        # [O, G, 64]
    OH = O_CORE // N_OPASS
    wns, sts, bts = [], [], []
    for c in range(N_CORES):
        sl = slice(c * O_CORE, (c + 1) * O_CORE)
        uc = u[sl].transpose(1, 2, 0)                        # [128, 64, Oc]
        wn = np.empty((N_OPASS, NG, GS, OH), dtype=np.uint8)
        for p in range(N_OPASS):
            wn[p] = uc[:, :, p * OH:(p + 1) * OH]
        wns.append(wn)
        sts.append(np.ascontiguousarray(w_scale[sl, :, 0].T))        # [128,Oc] f32
        bts.append(np.ascontiguousarray(w_bias[sl, :, 0].T).astype(ml_dtypes.bfloat16))
    return wns, sts, bts


def build():
    """Build the per-core bass program (identical on all cores)."""
    NOJ = O_CORE // 128
    OPP = NOJ // N_OPASS
    OH = OPP * 128

    # ramped DMA chunk sizes: small first chunks so the PE starts early
    XCH = [1, 2, 4, 6] + [8] * 6 + [4]    # x k-tile chunks (sum 65; xsum first)
    WCH = [2, 2, 4] + [8] * 7             # weight k-tile chunks per pass (sum 64)

    nc = bacc.Bacc("TRN2", target_bir_lowering=False)
    xt_d = nc.dram_tensor("xt", [NG, NK, T], mybir.dt.bfloat16, kind="ExternalInput")
    wn_d = nc.dram_tensor("wn", [N_OPASS, NG, GS, OH], mybir.dt.uint8,
                          kind="ExternalInput")
    st_d = nc.dram_tensor("st", [NG, O_CORE], mybir.dt.float32, kind="ExternalInput")
    bt_d = nc.dram_tensor("bt", [NG, O_CORE], mybir.dt.bfloat16, kind="ExternalInput")
    yt_d = nc.dram_tensor("yt", [O_CORE, T], mybir.dt.float32,
                          kind="ExternalOutput")

    with tile.TileContext(nc) as tc:
        with (
            tc.tile_pool(name="resident", bufs=1) as rpool,
            tc.tile_pool(name="nibs", bufs=4) as bpool,
            tc.tile_pool(name="wts", bufs=6) as wpool,
            tc.tile_pool(name="psum", bufs=8, space="PSUM") as ppool,
        ):
            # bias on the vector engine's queue (feeds the opening bias
            # matmuls); scale halves on the scalar engine's queue
            bt_s = rpool.tile([NG, O_CORE], mybir.dt.bfloat16)
            nc.gpsimd.dma_start(bt_s[:], bt_d[:])
            st_s = rpool.tile([NG, O_CORE], mybir.dt.float32)
            for p in range(N_OPASS):
                nc.scalar.dma_start(st_s[:, p * OH:(p + 1) * OH],
                                    st_d[:, p * OH:(p + 1) * OH])
            # x on the gpsimd engine's queue, ramped chunks
            xt_s = rpool.tile([NG, NK, T], mybir.dt.bfloat16)
            k0 = 0
            for ch in XCH:
                nc.gpsimd.dma_start(xt_s[:, k0:k0 + ch, :], xt_d[:, k0:k0 + ch, :])
                k0 += ch

            for p in range(N_OPASS):
                oo = p * OH
                psums = [ppool.tile([128, T], mybir.dt.float32, tag="ps",
                                    name=f"ps_{p}_{j}")
                         for j in range(OPP)]
                # bias k-tile first: needs only xsum (xt idx 0) + bt
                for j in range(OPP):
                    nc.tensor.matmul(
                        psums[j][:],
                        bt_s[:, oo + j * 128: oo + (j + 1) * 128],
                        xt_s[:, 0, :],
                        start=True, stop=False)
                k0 = 0
                for ch in WCH:
                    # weights on the sync engine's queue, chunked
                    nt = bpool.tile([NG, ch, OH], mybir.dt.uint8, tag="nib",
                                    name=f"nib_{p}_{k0}")
                    nc.sync.dma_start(nt[:], wn_d[p, :, k0:k0 + ch, :])
                    for kk in range(ch):
                        k = k0 + kk
                        wt = wpool.tile([NG, OH], mybir.dt.bfloat16, tag="wt")
                        nc.vector.tensor_mul(wt[:], nt[:, kk, :],
                                             st_s[:, oo:oo + OH])
                        for j in range(OPP):
                            nc.tensor.matmul(
                                psums[j][:],
                                wt[:, j * 128:(j + 1) * 128],
                                xt_s[:, k + 1, :],
                                start=False, stop=(k == GS - 1))
                    k0 += ch
                for j in range(OPP):
                    ot = wpool.tile([128, T], mybir.dt.float32, tag="ot")
                    nc.vector.tensor_copy(ot[:], psums[j][:])
                    nc.scalar.dma_start(
                        yt_d[oo + j * 128: oo + (j + 1) * 128, :], ot[:])

    nc.compile()
    return nc


_NC_CACHE = None


def get_nc():
    global _NC_CACHE
    if _NC_CACHE is None:
        _NC_CACHE = build()
    return _NC_CACHE


def make_in_maps(x, w_packed, w_scale, w_bias):
    xt = host_prep_x(np.asarray(x, dtype=np.float32))
    wns, sts, bts = host_prep_w(np.asarray(w_packed), np.asarray(w_scale),
                                np.asarray(w_bias))
    return [{"xt": xt, "wn": wns[c], "st": sts[c], "bt": bts[c]}
            for c in range(N_CORES)]


def assemble_out(results):
    yt = np.concatenate([np.asarray(r["yt"]) for r in results], axis=0)
    return np.ascontiguousarray(yt.T).reshape(B, S, OUT_F).astype(np.float32)


def run(x, w_packed, w_scale, w_bias, trace=False, **kw):
    nc = get_nc()
    in_maps = make_in_maps(x, w_packed, w_scale, w_bias)
    res = bass_utils.run_bass_kernel_spmd(
        nc, in_maps, core_ids=list(range(N_CORES)), trace=trace, **kw)
    return assemble_out(res.results), res


def kernel(x, w_packed, w_scale, w_bias):
    out, _ = run(x, w_packed, w_scale, w_bias, trace=False)
    return out
